# revision 1
# baseline (speedup 1.0000x reference)
"""Trainium2 Bass kernel for AstraloraLayer: y = (quantize(x) @ quantize(W).T) * scale.

Data-parallel across 8 NeuronCores: x sharded along the flattened token axis;
w (4 MB) and scale replicated; no collectives.

Per-core device program (shapes after host-side transposes):
  x    : [1024, 4096]  f32   x^T shard  (d_inp, tokens)
  w    : [1024, 1024]  f32   W^T        (d_inp, d_out)
  scale: [1]           f32
  out  : [1024, 4096]  bf16  y^T shard  (d_out, tokens); host upcasts to f32

Scheme:
  quantize(v, vmin, vmax, 8): q = round((clip(v)-vmin)/step), vq = q*step+vmin
  - round() = fp32 round-to-nearest-even via +-1.5*2^23 magic (matches
    jnp.round half-to-even).
  - x: ACT affine (-> round target for q-128), two DVE dual-op passes
    (round+clamp; 2D contiguous APs keep DVE in 2x fp32 mode), ACT affine
    back to [-3,3] + bf16 cast. Tile 0 in quarters so the PE starts ~10us.
  - w: three DVE dual-op passes (affine, round, scale-affine) with
    `scale` folded in (wq' = scale*wq, bf16). Clamps provably inactive
    for 0.02*randn weights.
  - scale broadcast via a K=1 f32 matmul (avoids the GPSIMD
    partition_broadcast library load, ~10us).
  - PE: y^T = wq' stationary @ xq moving; two 4-bank PSUM groups per token
    tile; tile 0 runs c-outer so matmuls chase the w/x quantize streams.
  - Choreography: the full quantize chain of tile t+1 is emitted before
    tile t's matmuls; psum->bf16 group copies alternate between ACT and
    DVE; out-DMAs ride GPSIMD's SWDGE so the Sync HWDGE FIFO only
    carries input prefetch. Last tile drains in four 2-bank groups to
    shorten the tail. (This exact emission order measured fastest; the
    Tile scheduler is highly sensitive to queue FIFO order.)
"""

import numpy as np

import concourse.bass as bass
import concourse.tile as tile
from concourse import bacc, mybir
from concourse.bass_utils import run_bass_kernel_spmd

F32 = mybir.dt.float32
BF16 = mybir.dt.bfloat16

N_CORES = 8
D = 1024
N_TOK = 16 * 2048
TOK_PER_CORE = N_TOK // N_CORES  # 4096
TT = 512  # token tile (PSUM bank = 512 f32)
N_TTILES = TOK_PER_CORE // TT  # 8
NCH = D // 128  # 8 chunks of 128 along d_inp / d_out

MAGIC = np.float32(1.5 * 2.0**23)  # v+MAGIC stays in [2^23, 2^24): ulp = 1

# x quantization constants (X_MIN=-3, X_MAX=3, 8 bits)
SX = np.float32(np.float32(6.0) / np.float32(255.0))
INV_SX = np.float32(42.5)  # 255/6, exact
HX = np.float32(np.float32(128.0) * SX + np.float32(-3.0))

# w quantization constants (W_MIN=-0.2, W_MAX=0.2, 8 bits)
SW = np.float32(np.float32(0.4) / np.float32(255.0))
INV_SW = np.float32(637.5)  # 255/0.4, exact
HW_OFF = np.float32(np.float32(128.0) * SW + np.float32(-0.2))

add = mybir.AluOpType.add
mult = mybir.AluOpType.mult
amax = mybir.AluOpType.max
amin = mybir.AluOpType.min


def build_nc():
    nc = bacc.Bacc(
        "TRN2",
        target_bir_lowering=False,
        debug=False,
        num_devices=N_CORES,
    )
    x = nc.dram_tensor("x", [D, TOK_PER_CORE], F32, kind="ExternalInput")
    w = nc.dram_tensor("w", [D, D], F32, kind="ExternalInput")
    scale = nc.dram_tensor("scale", [1], F32, kind="ExternalInput")
    out = nc.dram_tensor("out", [D, TOK_PER_CORE], BF16, kind="ExternalOutput")

    x_pct = x.rearrange("(c p) t -> p c t", p=128)  # [128, 8, 4096]
    w_pco = w.rearrange("(c p) o -> p c o", p=128)  # [128, 8, 1024]
    out_pct = out.rearrange("(c p) t -> p c t", p=128)  # [128, 8, 4096]

    COPY = mybir.ActivationFunctionType.Copy

    with tile.TileContext(nc) as tc:
        with (
            tc.tile_pool(name="wstage", bufs=3) as wstage_pool,
            tc.tile_pool(name="wq", bufs=1) as wq_pool,
            tc.tile_pool(name="consts", bufs=1) as const_pool,
            tc.tile_pool(name="xstage", bufs=3) as xstage_pool,
            tc.tile_pool(name="xq", bufs=3) as xq_pool,
            tc.tile_pool(name="outsb", bufs=4) as out_pool,
            tc.tile_pool(name="psum", bufs=2, space="PSUM") as psum_pool,
        ):
            # ---- scale broadcast (K=1 matmul) + PE warmup ------------------
            sc_one = const_pool.tile([1, 1], F32)
            ones_row = const_pool.tile([1, 128], F32)
            nc.gpsimd.memset(ones_row[:], 1.0)
            warm_lhs = const_pool.tile([128, 128], BF16)
            nc.gpsimd.memset(warm_lhs[:], 0.0)
            sw_sc = const_pool.tile([128, 1], F32)  # scale*SW
            hw_sc = const_pool.tile([128, 1], F32)  # scale*HW_OFF

            def scale_prep():
                nc.sync.dma_start(out=sc_one[:], in_=scale[0:1])
                ps_sc = psum_pool.tile([128, 4 * TT], F32, tag="ps")
                nc.tensor.matmul(ps_sc[:, 0:1], ones_row[:], sc_one[:], start=True, stop=True)
                nc.vector.tensor_scalar(sw_sc[:], ps_sc[:, 0:1], float(SW), None, mult)
                nc.vector.tensor_scalar(hw_sc[:], ps_sc[:, 0:1], float(HW_OFF), None, mult)

            # ---- W path: wq' = scale*quantize(w) in bf16, [128, 8192] flat -
            wq = wq_pool.tile([128, NCH * D], BF16)

            def w_prep(c):
                wst = wstage_pool.tile([128, D], F32, tag="wst")
                nc.sync.dma_start(out=wst[:], in_=w_pco[:, c, :])
                # v = w*637.5 - 0.5 (round target for qw-128; clamps inactive)
                nc.vector.tensor_scalar(wst[:], wst[:], float(INV_SW), -0.5, mult, add)
                # round to nearest-even
                nc.vector.tensor_scalar(wst[:], wst[:], float(MAGIC), -float(MAGIC), add, add)
                # wq' = scale * ((qw-128)*SW + HW_OFF)  -> bf16
                nc.vector.tensor_scalar(
                    wq[:, bass.ts(c, D)], wst[:], sw_sc[:], hw_sc[:], mult, add
                )

            def x_quant(xst, xq_t, sl):
                # v = x*42.5 - 0.5 (ACT fma; round target for q-128)
                nc.scalar.activation(xst[:, sl], xst[:, sl], COPY, bias=-0.5, scale=float(INV_SX))
                # round + lower clamp: u = max(rne(v+M), M-128)
                nc.vector.tensor_scalar(
                    xst[:, sl], xst[:, sl], float(MAGIC), float(MAGIC) - 128.0, add, amax
                )
                # upper clamp + unshift: r = min(u, M+127) - M (exact ints)
                nc.vector.tensor_scalar(
                    xst[:, sl], xst[:, sl], float(MAGIC) + 127.0, -float(MAGIC), amin, add
                )
                # xq = r*SX + HX -> bf16
                nc.scalar.activation(xq_t[:, sl], xst[:, sl], COPY, bias=float(HX), scale=float(SX))

            copy_ctr = [0]

            def matmul_group(t, xq_t, grp, c_outer):
                ng = len(grp)
                ps = psum_pool.tile([128, 4 * TT], F32, tag="ps")
                if c_outer:
                    order = [(c, oo) for c in range(NCH) for oo in range(ng)]
                else:
                    order = [(c, oo) for oo in range(ng) for c in range(NCH)]
                for c, oo in order:
                    o = grp[oo]
                    nc.tensor.matmul(
                        ps[:, bass.ts(oo, TT)],
                        wq[:, c * D + o * 128 : c * D + o * 128 + 128],
                        xq_t[:, bass.ts(c, TT)],
                        start=(c == 0), stop=(c == NCH - 1),
                    )
                osb = out_pool.tile([128, ng, TT], BF16, tag=f"osb{ng}")
                if copy_ctr[0] % 2 == 0:
                    nc.scalar.copy(osb[:], ps[:, : ng * TT])
                else:
                    nc.vector.tensor_copy(osb[:], ps[:, : ng * TT])
                copy_ctr[0] += 1
                # SWDGE: keeps the Sync HWDGE FIFO free for input prefetch
                nc.gpsimd.dma_start(
                    out=out_pct[:, grp[0] : grp[0] + ng, bass.ts(t, TT)],
                    in_=osb[:],
                )

            # ---- tile 0: quarters; x0 chain gets queue priority ------------
            xst0 = xstage_pool.tile([128, NCH * TT], F32, tag="xst")
            xq0 = xq_pool.tile([128, NCH * TT], BF16, tag="xq")
            Q = 2 * TT  # quarter = 2 c-chunks

            def q_sl(i):
                return slice(i * Q, (i + 1) * Q)

            nc.sync.dma_start(out=xst0[:, q_sl(0)], in_=x_pct[:, 0:2, bass.ts(0, TT)])
            nc.sync.dma_start(out=xst0[:, q_sl(1)], in_=x_pct[:, 2:4, bass.ts(0, TT)])
            scale_prep()
            x_quant(xst0, xq0, q_sl(0))
            w_prep(0)
            w_prep(1)
            nc.sync.dma_start(out=xst0[:, q_sl(2)], in_=x_pct[:, 4:6, bass.ts(0, TT)])
            x_quant(xst0, xq0, q_sl(1))
            w_prep(2)
            w_prep(3)
            nc.sync.dma_start(out=xst0[:, q_sl(3)], in_=x_pct[:, 6:8, bass.ts(0, TT)])
            x_quant(xst0, xq0, q_sl(2))
            w_prep(4)
            w_prep(5)
            x_quant(xst0, xq0, q_sl(3))
            w_prep(6)
            w_prep(7)

            # ---- steady tiles, software-pipelined in program order ---------
            def x_prep(t):
                xst = xstage_pool.tile([128, NCH * TT], F32, tag="xst")
                nc.sync.dma_start(out=xst[:], in_=x_pct[:, :, bass.ts(t, TT)])
                xq_t = xq_pool.tile([128, NCH * TT], BF16, tag="xq")
                x_quant(xst, xq_t, slice(None))
                return xq_t

            xq_next = x_prep(1)
            matmul_group(0, xq0, [0, 1, 2, 3], c_outer=True)
            matmul_group(0, xq0, [4, 5, 6, 7], c_outer=True)
            for t in range(1, N_TTILES):
                xq_cur = xq_next
                if t + 1 < N_TTILES:
                    xq_next = x_prep(t + 1)
                if t < N_TTILES - 1:
                    matmul_group(t, xq_cur, [0, 1, 2, 3], False)
                    matmul_group(t, xq_cur, [4, 5, 6, 7], False)
                else:  # last tile: finer drain groups to shorten the tail
                    matmul_group(t, xq_cur, [0, 1], False)
                    matmul_group(t, xq_cur, [2, 3], False)
                    matmul_group(t, xq_cur, [4, 5], False)
                    matmul_group(t, xq_cur, [6, 7], False)

    nc.compile()
    return nc


def _shard_inputs(x, w, scale):
    x = np.ascontiguousarray(np.asarray(x, dtype=np.float32))
    w = np.ascontiguousarray(np.asarray(w, dtype=np.float32))
    scale = np.ascontiguousarray(np.asarray(scale, dtype=np.float32))
    xT = np.ascontiguousarray(x.reshape(N_TOK, D).T)  # [1024, 32768]
    wT = np.ascontiguousarray(w.reshape(D, D).T)  # [i, o]
    in_maps = []
    for k in range(N_CORES):
        in_maps.append(
            {
                "x": np.ascontiguousarray(
                    xT[:, k * TOK_PER_CORE : (k + 1) * TOK_PER_CORE]
                ),
                "w": wT,
                "scale": scale,
            }
        )
    return in_maps


def _gather_output(results):
    yT = np.concatenate(
        [np.asarray(results[k]["out"], dtype=np.float32) for k in range(N_CORES)],
        axis=1,
    )  # [1024, 32768] f32
    return np.ascontiguousarray(yT.T).reshape(16, 2048, D)


def run(x, w, scale, trace=False, **run_kwargs):
    """Build + run on the 8 NeuronCores; returns (output, BassKernelResults)."""
    in_maps = _shard_inputs(x, w, scale)
    nc = build_nc()
    res = run_bass_kernel_spmd(
        nc, in_maps, core_ids=list(range(N_CORES)), trace=trace, **run_kwargs
    )
    return _gather_output(res.results), res


def kernel(x, w, scale):
    out, _ = run(x, w, scale, trace=False)
    return out



# revision 4
# speedup vs baseline: 1.0335x; 1.0335x over previous
"""Trainium2 Bass kernel for AstraloraLayer: y = (quantize(x) @ quantize(W).T) * scale.

Data-parallel across 8 NeuronCores: x sharded along the flattened token axis;
quantized weights replicated; no collectives.

Math: both quantizers are mid-rise: xq = SX*(ix+0.5), wq = SW*(iw+0.5) with
ix, iw in [-128, 127] (SX=6/255, SW=0.4/255; 128*SX-3 = SX/2 exactly).
  y[n,o] = scale * sum_k xq wq = sum_k W''[k,o] * ix[k,n] + beta[o]
with W'' = scale*SX*wq (bf16, host-precomputed mirror of the reference f32
quantizer) and beta[o] = 0.5 * sum_k bf16(W''[k,o]) (added on host).
So the device moving operand is the *integer* ix in bf16 (exact), and the
whole quantize+scale affine collapses into the weights.

Host ships xs = fp16(x*42.5 - 0.5); device x-quantize is just 2 DVE passes:
  t  = max(rne(xs + M), M-128)     (magic M = 1.5*2^23: +M rounds to int)
  ix = min(t, M+127) - M  -> bf16  (clamp + unshift, exact integers)

Per-core device program (4096 tokens, 8 token tiles of 512):
  xs  : [8][128, 4096] fp16  tile-contiguous (8 KB/partition-line DMAs)
  wq2 : [8][128, 1024] bf16  k-chunk-contiguous W''
  out : [8][128, 8, 512] bf16 (o-part within group, o-group, token)
Per tile: 64 matmuls (8 k-chunks x 8 o-groups) into 2 psum groups of 4 banks,
evacuated by one ACT copy each (ACT reads PSUM fast), out-DMA per group.

DMA routing: x prefetch on the Sync HWDGE queue; weights + all output on the
Scalar-engine HWDGE queue (hardware descriptors; the v1 SWDGE output path ran
at ~85 GB/s and made the last tile drain ~12 us).
A few warmup matmuls on a scratch tile keep the PE HAM clock ramping while
the first x tile lands.
"""

import numpy as np

try:
    from ml_dtypes import bfloat16 as np_bf16
except ImportError:  # pragma: no cover
    np_bf16 = None

import concourse.bass as bass
import concourse.tile as tile
from concourse import bacc, mybir
from concourse.bass_utils import run_bass_kernel_spmd

F32 = mybir.dt.float32
F16 = mybir.dt.float16
BF16 = mybir.dt.bfloat16

N_CORES = 8
D = 1024
N_TOK = 16 * 2048
TOK_PER_CORE = N_TOK // N_CORES  # 4096
TT = 512  # token tile (PSUM bank = 512 f32)
N_TTILES = TOK_PER_CORE // TT  # 8
NCH = D // 128  # 8 k-chunks / o-groups

MAGIC = np.float32(1.5 * 2.0**23)  # v+MAGIC stays in [2^23, 2^24): ulp = 1

SX = np.float32(np.float32(6.0) / np.float32(255.0))
INV_SX = np.float32(42.5)  # 255/6, exact
SW = np.float32(np.float32(0.4) / np.float32(255.0))
INV_SW = np.float32(637.5)  # 255/0.4, exact

add = mybir.AluOpType.add
amax = mybir.AluOpType.max
amin = mybir.AluOpType.min


def build_nc():
    nc = bacc.Bacc(
        "TRN2",
        target_bir_lowering=False,
        debug=False,
        num_devices=N_CORES,
    )
    xs_d = nc.dram_tensor("xs", [N_TTILES, 128, NCH * TT], F16, kind="ExternalInput")
    wq_d = nc.dram_tensor("wq", [NCH, 128, D], BF16, kind="ExternalInput")
    out_d = nc.dram_tensor("out", [N_TTILES, 128, NCH, TT], BF16, kind="ExternalOutput")

    with tile.TileContext(nc) as tc:
        with (
            tc.tile_pool(name="wq", bufs=1) as wq_pool,
            tc.tile_pool(name="consts", bufs=1) as const_pool,
            tc.tile_pool(name="xs", bufs=3) as xs_pool,
            tc.tile_pool(name="tst", bufs=2) as tst_pool,
            tc.tile_pool(name="xi", bufs=3) as xi_pool,
            tc.tile_pool(name="outsb", bufs=4) as out_pool,
            tc.tile_pool(name="psum", bufs=2, space="PSUM") as psum_pool,
        ):
            # ---- weights on the Scalar HWDGE queue (one chunk = one DMA) ---
            wq_t = wq_pool.tile([128, NCH * D], BF16)
            nc.scalar.dma_start(out=wq_t[:, bass.ts(0, D)], in_=wq_d[0])

            # ---- PE warmup: keep HAM ramping while first x tile lands ------
            warm = const_pool.tile([128, TT], BF16)
            nc.vector.memset(warm[:], 0.0)
            ps_w = psum_pool.tile([128, 4 * TT], F32, tag="ps")
            for i in range(8):
                nc.tensor.matmul(
                    ps_w[:, 0:TT], warm[:, 0:128], warm[:],
                    start=(i == 0), stop=(i == 7),
                )

            for c in range(1, NCH):
                nc.scalar.dma_start(out=wq_t[:, bass.ts(c, D)], in_=wq_d[c])

            # ---- x quantize: 2 DVE passes -> exact integer bf16 ------------
            def x_quant(xs_t, xi_t, sl):
                t_t = tst_pool.tile([128, NCH * TT], F32, tag="tst")
                nc.vector.tensor_scalar(
                    t_t[:, sl], xs_t[:, sl], float(MAGIC), float(MAGIC) - 128.0,
                    add, amax,
                )
                nc.vector.tensor_scalar(
                    xi_t[:, sl], t_t[:, sl], float(MAGIC) + 127.0, -float(MAGIC),
                    amin, add,
                )

            def matmul_group(t, xi_t, grp, c_outer, drain_halves=False):
                ng = len(grp)
                ps = psum_pool.tile([128, 4 * TT], F32, tag="ps")
                if c_outer:
                    order = [(c, oo) for c in range(NCH) for oo in range(ng)]
                else:
                    order = [(c, oo) for oo in range(ng) for c in range(NCH)]
                for c, oo in order:
                    o = grp[oo]
                    nc.tensor.matmul(
                        ps[:, bass.ts(oo, TT)],
                        wq_t[:, c * D + o * 128 : c * D + o * 128 + 128],
                        xi_t[:, bass.ts(c, TT)],
                        start=(c == 0), stop=(c == NCH - 1),
                    )
                if not drain_halves:
                    osb = out_pool.tile([128, ng, TT], BF16, tag=f"osb{ng}")
                    nc.scalar.copy(osb[:], ps[:, : ng * TT])
                    nc.scalar.dma_start(
                        out=out_d[t, :, grp[0] : grp[0] + ng, :], in_=osb[:]
                    )
                else:  # finer tail: evacuate + DMA two banks at a time
                    h = ng // 2
                    for j in range(2):
                        osb = out_pool.tile([128, h, TT], BF16, tag=f"osbh{j}")
                        nc.scalar.copy(osb[:], ps[:, j * h * TT : (j + 1) * h * TT])
                        nc.scalar.dma_start(
                            out=out_d[t, :, grp[j * h] : grp[j * h] + h, :],
                            in_=osb[:],
                        )

            # ---- tile 0 in quarters so the PE starts ASAP ------------------
            xs0 = xs_pool.tile([128, NCH * TT], F16, tag="xs")
            xi0 = xi_pool.tile([128, NCH * TT], BF16, tag="xi")
            Q = 2 * TT  # quarter = 2 k-chunks

            def q_sl(i):
                return slice(i * Q, (i + 1) * Q)

            for i in range(4):
                nc.sync.dma_start(
                    out=xs0[:, q_sl(i)], in_=xs_d[0, :, q_sl(i)]
                )
                x_quant(xs0, xi0, q_sl(i))

            def x_prep(t):
                xs_t = xs_pool.tile([128, NCH * TT], F16, tag="xs")
                nc.sync.dma_start(out=xs_t[:], in_=xs_d[t])
                xi_t = xi_pool.tile([128, NCH * TT], BF16, tag="xi")
                x_quant(xs_t, xi_t, slice(None))
                return xi_t

            xq_next = x_prep(1)
            matmul_group(0, xi0, [0, 1, 2, 3], c_outer=True)
            matmul_group(0, xi0, [4, 5, 6, 7], c_outer=True)
            for t in range(1, N_TTILES):
                xq_cur = xq_next
                if t + 1 < N_TTILES:
                    xq_next = x_prep(t + 1)
                last = t == N_TTILES - 1
                matmul_group(t, xq_cur, [0, 1, 2, 3], False, drain_halves=last)
                matmul_group(t, xq_cur, [4, 5, 6, 7], False, drain_halves=last)

    nc.compile()
    return nc


def _quantize_w_host(w, scale):
    """Mirror of the reference f32 quantizer for w, folded with scale*SX.

    Returns (wq2 bf16 [NCH,128,D] chunk-major k x o, beta f32 [1024])."""
    w = np.asarray(w, dtype=np.float32)
    levels = np.float32(2.0**8 - 1.0)
    step = (np.float32(0.2) - np.float32(-0.2)) / levels
    q = np.clip(w, np.float32(-0.2), np.float32(0.2))
    q = np.round((q - np.float32(-0.2)) / step).astype(np.float32)
    wq = q * step + np.float32(-0.2)  # reference-exact f32 quantized w
    s = np.float32(np.float32(np.asarray(scale, dtype=np.float32).ravel()[0]) * SX)
    w2 = (s * wq).reshape(D, D)  # [o, i]
    w2T = np.ascontiguousarray(w2.T)  # [i, o]
    w2T_bf = w2T.astype(np_bf16)
    beta = 0.5 * w2T_bf.astype(np.float64).sum(axis=0)  # [o]
    wq2 = np.ascontiguousarray(w2T_bf.reshape(NCH, 128, D))
    return wq2, beta.astype(np.float32)


def _prep_inputs(x, w, scale):
    x = np.asarray(x, dtype=np.float32).reshape(N_TOK, D)
    xs = (x * INV_SX - np.float32(0.5)).astype(np.float16)
    wq2, beta = _quantize_w_host(w, scale)
    in_maps = []
    for k in range(N_CORES):
        xk = xs[k * TOK_PER_CORE : (k + 1) * TOK_PER_CORE]  # [4096, 1024]
        # [t, tt, c, p] -> [t, p, c, tt]
        xk = xk.reshape(N_TTILES, TT, NCH, 128).transpose(0, 3, 2, 1)
        in_maps.append(
            {
                "xs": np.ascontiguousarray(xk.reshape(N_TTILES, 128, NCH * TT)),
                "wq": wq2,
            }
        )
    return in_maps, beta


def _gather_output(results, beta):
    parts = []
    for k in range(N_CORES):
        o = np.asarray(results[k]["out"]).astype(np.float32)  # [t, p, og, tt]
        # -> [t, tt, og, p] -> [4096, 1024]
        o = o.transpose(0, 3, 2, 1).reshape(TOK_PER_CORE, D)
        parts.append(o)
    y = np.concatenate(parts, axis=0)  # [32768, 1024]
    y += beta[None, :]
    return y.reshape(16, 2048, D)


def run(x, w, scale, trace=False, **run_kwargs):
    """Build + run on the 8 NeuronCores; returns (output, BassKernelResults)."""
    in_maps, beta = _prep_inputs(x, w, scale)
    nc = build_nc()
    res = run_bass_kernel_spmd(
        nc, in_maps, core_ids=list(range(N_CORES)), trace=trace, **run_kwargs
    )
    return _gather_output(res.results, beta), res


def kernel(x, w, scale):
    out, _ = run(x, w, scale, trace=False)
    return out


# revision 5
# speedup vs baseline: 1.1092x; 1.0732x over previous
"""Trainium2 Bass kernel for AstraloraLayer: y = (quantize(x) @ quantize(W).T) * scale.

Data-parallel across 8 NeuronCores: x sharded along the flattened token axis;
quantized weights replicated; no collectives.

Math: both quantizers are mid-rise: xq = SX*(ix+0.5), wq = SW*(iw+0.5) with
ix, iw in [-128, 127] (SX=6/255, SW=0.4/255; 128*SX-3 = SX/2 exactly).
  y[n,o] = scale * sum_k xq wq = sum_k W''[k,o] * ix[k,n] + beta[o]
with W'' = scale*SX*wq (bf16, host-precomputed mirror of the reference f32
quantizer) and beta[o] = 0.5 * sum_k bf16(W''[k,o]) (added on host).
The device moving operand is the *integer* ix in bf16 (exact); the whole
quantize+scale affine collapses into the weights.

Host ships xs = fp16(x*42.5 - 0.5); device x-quantize is 2 DVE passes in the
fp16 magic domain (M16 = 1536: ulp(v+1536) = 1 for |v| <= 511, so the fp16
output cast rounds to integer, RNE like jnp.round):
  t  = max(xs + 1536, 1408)          -> fp16 (cast rounds; 1408 = -128 clamp)
  ix = min(t, 1663) - 1536           -> bf16 exact integers
All-16-bit passes keep the DVE in its 2x mode.

Per-core device program (4096 tokens, 8 token tiles of 512):
  xs  : [8][128, 4096] fp16  tile-contiguous (8 KB/partition-line DMAs)
  wq  : [8][128, 1024] bf16  k-chunk-contiguous W''
  out : [8][4][128, 2, 512] bf16 (tile, og-pair, part, og, token) - every
        out-DMA writes one fully contiguous 256 KB block.
Per tile: 64 matmuls (og-outer, 8 k-chunks each) into 2 psum groups of 4
banks; per-og evacuation alternates ACT/DVE so the tail pair drains in
parallel; out-DMA per og-pair on the Scalar HWDGE queue.

Startup choreography: 8 warmup matmuls on a zero tile ramp the PE HAM clock
while the first x quarter lands; tile 0 is DMAd/quantized in quarters and
multiplied c-outer so the PE chases the quantize stream; tiles 1-2 are
DMAd/quantized in halves to stay ahead of the warm 216 ns/MM cadence.
x prefetch rides the Sync HWDGE queue; weights + output the Scalar queue.
"""

import numpy as np

try:
    from ml_dtypes import bfloat16 as np_bf16
except ImportError:  # pragma: no cover
    np_bf16 = None

import concourse.bass as bass
import concourse.tile as tile
from concourse import bacc, mybir
from concourse.bass_utils import run_bass_kernel_spmd

F32 = mybir.dt.float32
F16 = mybir.dt.float16
BF16 = mybir.dt.bfloat16

N_CORES = 8
D = 1024
N_TOK = 16 * 2048
TOK_PER_CORE = N_TOK // N_CORES  # 4096
TT = 512  # token tile (PSUM bank = 512 f32)
N_TTILES = TOK_PER_CORE // TT  # 8
NCH = D // 128  # 8 k-chunks / o-groups

M16 = 1536.0  # fp16 magic: ulp = 1 on [1024, 2048)

SX = np.float32(np.float32(6.0) / np.float32(255.0))
INV_SX = np.float32(42.5)  # 255/6, exact

add = mybir.AluOpType.add
amax = mybir.AluOpType.max
amin = mybir.AluOpType.min


def build_nc():
    nc = bacc.Bacc(
        "TRN2",
        target_bir_lowering=False,
        debug=False,
        num_devices=N_CORES,
    )
    xs_d = nc.dram_tensor("xs", [N_TTILES, 128, NCH * TT], F16, kind="ExternalInput")
    wq_d = nc.dram_tensor("wq", [NCH, 128, D], BF16, kind="ExternalInput")
    out_d = nc.dram_tensor(
        "out", [N_TTILES, NCH // 2, 128, 2, TT], BF16, kind="ExternalOutput"
    )

    with tile.TileContext(nc) as tc:
        with (
            tc.tile_pool(name="wq", bufs=1) as wq_pool,
            tc.tile_pool(name="consts", bufs=1) as const_pool,
            tc.tile_pool(name="xs", bufs=3) as xs_pool,
            tc.tile_pool(name="tst", bufs=2) as tst_pool,
            tc.tile_pool(name="xi", bufs=3) as xi_pool,
            tc.tile_pool(name="outsb", bufs=6) as out_pool,
            tc.tile_pool(name="psum", bufs=2, space="PSUM") as psum_pool,
        ):
            # ---- weights on the Scalar HWDGE queue (one chunk = one DMA) ---
            wq_t = wq_pool.tile([128, NCH * D], BF16)
            for c in range(NCH):
                nc.scalar.dma_start(out=wq_t[:, bass.ts(c, D)], in_=wq_d[c])

            # ---- PE warmup: ramp the HAM clock while first x tile lands ----
            warm = const_pool.tile([128, TT], BF16)
            nc.vector.memset(warm[:], 0.0)
            ps_w = psum_pool.tile([128, 4 * TT], F32, tag="ps")
            for i in range(8):
                nc.tensor.matmul(
                    ps_w[:, 0:TT], warm[:, 0:128], warm[:],
                    start=(i == 0), stop=(i == 7),
                )
            # preload the ACT Copy table so the first evac doesn't stall
            warm16 = const_pool.tile([128, 1], BF16)
            nc.scalar.copy(warm16[:], warm[:, 0:1])

            # ---- x quantize: 2 DVE passes, all 16-bit (fp16 magic) ---------
            def x_quant(xs_t, xi_t, sl):
                t_t = tst_pool.tile([128, NCH * TT], F16, tag="tst")
                nc.vector.tensor_scalar(
                    t_t[:, sl], xs_t[:, sl], M16, M16 - 128.0, add, amax
                )
                nc.vector.tensor_scalar(
                    xi_t[:, sl], t_t[:, sl], M16 + 127.0, -M16, amin, add
                )

            def matmul_group(t, xi_t, grp, c_outer):
                ng = len(grp)
                ps = psum_pool.tile([128, 4 * TT], F32, tag="ps")
                if c_outer:
                    order = [(c, oo) for c in range(NCH) for oo in range(ng)]
                else:
                    order = [(c, oo) for oo in range(ng) for c in range(NCH)]
                for c, oo in order:
                    o = grp[oo]
                    nc.tensor.matmul(
                        ps[:, bass.ts(oo, TT)],
                        wq_t[:, c * D + o * 128 : c * D + o * 128 + 128],
                        xi_t[:, bass.ts(c, TT)],
                        start=(c == 0), stop=(c == NCH - 1),
                    )
                # per-og evacuation, ACT/DVE alternating; out-DMA per og-pair
                for j in range(0, ng, 2):
                    osb = out_pool.tile([128, 2, TT], BF16, tag="osb")
                    nc.scalar.copy(osb[:, 0, :], ps[:, bass.ts(j, TT)])
                    nc.vector.tensor_copy(osb[:, 1, :], ps[:, bass.ts(j + 1, TT)])
                    nc.scalar.dma_start(out=out_d[t, grp[j] // 2], in_=osb[:])

            # ---- tile 0 in quarters so the PE starts ASAP ------------------
            xs0 = xs_pool.tile([128, NCH * TT], F16, tag="xs")
            xi0 = xi_pool.tile([128, NCH * TT], BF16, tag="xi")
            Q = 2 * TT  # quarter = 2 k-chunks

            for i in range(4):
                sl = slice(i * Q, (i + 1) * Q)
                nc.sync.dma_start(out=xs0[:, sl], in_=xs_d[0, :, sl])
                x_quant(xs0, xi0, sl)

            def x_prep(t, halves):
                xs_t = xs_pool.tile([128, NCH * TT], F16, tag="xs")
                xi_t = xi_pool.tile([128, NCH * TT], BF16, tag="xi")
                if halves:
                    for i in range(2):
                        sl = slice(i * 4 * TT, (i + 1) * 4 * TT)
                        nc.sync.dma_start(out=xs_t[:, sl], in_=xs_d[t, :, sl])
                        x_quant(xs_t, xi_t, sl)
                else:
                    nc.sync.dma_start(out=xs_t[:], in_=xs_d[t])
                    x_quant(xs_t, xi_t, slice(None))
                return xi_t

            xq_next = x_prep(1, halves=True)
            matmul_group(0, xi0, [0, 1, 2, 3], c_outer=True)
            matmul_group(0, xi0, [4, 5, 6, 7], c_outer=True)
            for t in range(1, N_TTILES):
                xq_cur = xq_next
                if t + 1 < N_TTILES:
                    xq_next = x_prep(t + 1, halves=(t + 1 == 2))
                matmul_group(t, xq_cur, [0, 1, 2, 3], False)
                matmul_group(t, xq_cur, [4, 5, 6, 7], False)

    nc.compile()
    return nc


def _quantize_w_host(w, scale):
    """Mirror of the reference f32 quantizer for w, folded with scale*SX.

    Returns (wq2 bf16 [NCH,128,D] chunk-major k x o, beta f32 [1024])."""
    w = np.asarray(w, dtype=np.float32)
    levels = np.float32(2.0**8 - 1.0)
    step = (np.float32(0.2) - np.float32(-0.2)) / levels
    q = np.clip(w, np.float32(-0.2), np.float32(0.2))
    q = np.round((q - np.float32(-0.2)) / step).astype(np.float32)
    wq = q * step + np.float32(-0.2)  # reference-exact f32 quantized w
    s = np.float32(np.float32(np.asarray(scale, dtype=np.float32).ravel()[0]) * SX)
    w2 = (s * wq).reshape(D, D)  # [o, i]
    w2T = np.ascontiguousarray(w2.T)  # [i, o]
    w2T_bf = w2T.astype(np_bf16)
    beta = 0.5 * w2T_bf.astype(np.float64).sum(axis=0)  # [o]
    wq2 = np.ascontiguousarray(w2T_bf.reshape(NCH, 128, D))
    return wq2, beta.astype(np.float32)


def _prep_inputs(x, w, scale):
    x = np.asarray(x, dtype=np.float32).reshape(N_TOK, D)
    xs = (x * INV_SX - np.float32(0.5)).astype(np.float16)
    wq2, beta = _quantize_w_host(w, scale)
    in_maps = []
    for k in range(N_CORES):
        xk = xs[k * TOK_PER_CORE : (k + 1) * TOK_PER_CORE]  # [4096, 1024]
        # [t, tt, c, p] -> [t, p, c, tt]
        xk = xk.reshape(N_TTILES, TT, NCH, 128).transpose(0, 3, 2, 1)
        in_maps.append(
            {
                "xs": np.ascontiguousarray(xk.reshape(N_TTILES, 128, NCH * TT)),
                "wq": wq2,
            }
        )
    return in_maps, beta


def _gather_output(results, beta):
    parts = []
    for k in range(N_CORES):
        o = np.asarray(results[k]["out"]).astype(np.float32)  # [t, pair, p, og2, tt]
        # o[t, pair, p, og2, tt] -> y[t*512+tt, (pair*2+og2)*128 + p]
        o = o.transpose(0, 4, 1, 3, 2).reshape(TOK_PER_CORE, D)
        parts.append(o)
    y = np.concatenate(parts, axis=0)  # [32768, 1024]
    y += beta[None, :]
    return y.reshape(16, 2048, D)


def run(x, w, scale, trace=False, **run_kwargs):
    """Build + run on the 8 NeuronCores; returns (output, BassKernelResults)."""
    in_maps, beta = _prep_inputs(x, w, scale)
    nc = build_nc()
    res = run_bass_kernel_spmd(
        nc, in_maps, core_ids=list(range(N_CORES)), trace=trace, **run_kwargs
    )
    return _gather_output(res.results, beta), res


def kernel(x, w, scale):
    out, _ = run(x, w, scale, trace=False)
    return out


# revision 11
# speedup vs baseline: 1.1372x; 1.0253x over previous
"""Trainium2 Bass kernel for AstraloraLayer: y = (quantize(x) @ quantize(W).T) * scale.

Data-parallel across 8 NeuronCores: x sharded along the flattened token axis;
quantized weights replicated; no collectives.

Math: both quantizers are mid-rise: xq = SX*(ix+0.5), wq = SW*(iw+0.5) with
ix, iw in [-128, 127] (SX=6/255, SW=0.4/255; 128*SX-3 = SX/2 exactly).
  y[n,o] = scale * sum_k xq wq = sum_k W''[k,o] * ix[k,n] + beta[o]
with W'' = scale*SX*wq (bf16, host-precomputed mirror of the reference f32
quantizer) and beta[o] = 0.5 * sum_k bf16(W''[k,o]) (added on host).
The device moving operand is the *integer* ix in bf16 (exact); the whole
quantize+scale affine collapses into the weights.

Host ships xs = fp16(x*42.5 - 0.5); device x-quantize is 2 DVE passes in the
fp16 magic domain (M16 = 1536: ulp(v+1536) = 1 for |v| <= 511, so the fp16
output cast rounds to integer, RNE like jnp.round):
  t  = max(xs + 1536, 1408)          -> fp16 (cast rounds; 1408 = -128 clamp)
  ix = min(t, 1663) - 1536           -> bf16 exact integers
All-16-bit passes keep the DVE in its 2x mode.

Per-core device program (4096 tokens, 8 token tiles of 512):
  xs  : [8][128, 4096] fp16  tile-contiguous (8 KB/partition-line DMAs)
  wq  : [8][128, 1024] bf16  k-chunk-contiguous W''
  out : [8][4][128, 2, 512] bf16 (tile, og-pair, part, og, token) - every
        out-DMA writes one fully contiguous 256 KB block.
Per tile: 64 matmuls (og-outer, 8 k-chunks each) into 2 psum groups of 4
banks; per-og evacuation alternates ACT/DVE so the tail pair drains in
parallel; out-DMA per og-pair on the Scalar HWDGE queue.

Startup choreography: 8 warmup matmuls on a zero tile ramp the PE HAM clock
while the first x quarter lands; tile 0 is DMAd/quantized in quarters and
multiplied c-outer so the PE chases the quantize stream; tiles 1-2 are
DMAd/quantized in halves to stay ahead of the warm 216 ns/MM cadence.
x prefetch rides the Sync HWDGE queue; weights + output the Scalar queue.
"""

import numpy as np

try:
    from ml_dtypes import bfloat16 as np_bf16
except ImportError:  # pragma: no cover
    np_bf16 = None

import concourse.bass as bass
import concourse.tile as tile
from concourse import bacc, mybir
from concourse.bass_utils import run_bass_kernel_spmd

F32 = mybir.dt.float32
F16 = mybir.dt.float16
BF16 = mybir.dt.bfloat16

N_CORES = 8
D = 1024
N_TOK = 16 * 2048
TOK_PER_CORE = N_TOK // N_CORES  # 4096
TT = 512  # token tile (PSUM bank = 512 f32)
N_TTILES = TOK_PER_CORE // TT  # 8
NCH = D // 128  # 8 k-chunks / o-groups

M16 = 1536.0  # fp16 magic: ulp = 1 on [1024, 2048)

SX = np.float32(np.float32(6.0) / np.float32(255.0))
INV_SX = np.float32(42.5)  # 255/6, exact

add = mybir.AluOpType.add
amax = mybir.AluOpType.max
amin = mybir.AluOpType.min


def build_nc():
    nc = bacc.Bacc(
        "TRN2",
        target_bir_lowering=False,
        debug=False,
        num_devices=N_CORES,
    )
    xs_d = nc.dram_tensor("xs", [N_TTILES, 128, NCH * TT], F16, kind="ExternalInput")
    wq_d = nc.dram_tensor("wq", [NCH, 128, D], BF16, kind="ExternalInput")
    out_d = nc.dram_tensor(
        "out", [N_TTILES, NCH // 2, 128, 2, TT], BF16, kind="ExternalOutput"
    )

    with tile.TileContext(nc) as tc:
        with (
            tc.tile_pool(name="wq", bufs=1) as wq_pool,
            tc.tile_pool(name="consts", bufs=1) as const_pool,
            tc.tile_pool(name="xs", bufs=3) as xs_pool,
            tc.tile_pool(name="tst", bufs=2) as tst_pool,
            tc.tile_pool(name="xi", bufs=3) as xi_pool,
            tc.tile_pool(name="outsb", bufs=6) as out_pool,
            tc.tile_pool(name="psum", bufs=4, space="PSUM") as psum_pool,
        ):
            # ---- weights on the Scalar HWDGE queue (one chunk = one DMA) ---
            wq_t = wq_pool.tile([128, NCH * D], BF16)
            for c in range(NCH):
                nc.scalar.dma_start(out=wq_t[:, bass.ts(c, D)], in_=wq_d[c])

            # ---- PE warmup: ramp the HAM clock while first x tile lands ----
            warm = const_pool.tile([128, TT], BF16)
            nc.vector.memset(warm[:], 0.0)
            ps_w = psum_pool.tile([128, 2 * TT], F32, tag="ps")
            NWARM = 9
            for i in range(NWARM):
                nc.tensor.matmul(
                    ps_w[:, 0:TT], warm[:, 0:128], warm[:],
                    start=(i == 0), stop=(i == NWARM - 1),
                )
            # preload the ACT Copy table so the first evac doesn't stall
            warm16 = const_pool.tile([128, 1], BF16)
            nc.scalar.copy(warm16[:], warm[:, 0:1])

            # ---- x quantize: 2 DVE passes, all 16-bit (fp16 magic) ---------
            def x_quant(xs_t, xi_t, sl):
                t_t = tst_pool.tile([128, NCH * TT], F16, tag="tst")
                nc.vector.tensor_scalar(
                    t_t[:, sl], xs_t[:, sl], M16, M16 - 128.0, add, amax
                )
                nc.vector.tensor_scalar(
                    xi_t[:, sl], t_t[:, sl], M16 + 127.0, -M16, amin, add
                )

            def evac_pair(t, ps, pair):
                """Evacuate psum pair tile (og = 2*pair, 2*pair+1) and DMA out."""
                osb = out_pool.tile([128, 2, TT], BF16, tag="osb")
                nc.scalar.copy(osb[:, 0, :], ps[:, 0:TT])
                nc.vector.tensor_copy(osb[:, 1, :], ps[:, TT : 2 * TT])
                nc.scalar.dma_start(out=out_d[t, pair], in_=osb[:])

            def mm(ps, c, o, xi_t):
                nc.tensor.matmul(
                    ps[:, (o % 2) * TT : (o % 2) * TT + TT],
                    wq_t[:, c * D + o * 128 : c * D + o * 128 + 128],
                    xi_t[:, bass.ts(c, TT)],
                    start=(c == 0), stop=(c == NCH - 1),
                )

            def matmul_tile0(xi_t):
                # c-outer over ALL 8 og per chunk: one quantized chunk unlocks
                # 16 matmuls, so the PE chases the DMA/quantize stream.
                pstiles = []
                for _p in range(4):
                    ps0 = psum_pool.tile([128, 2 * TT], F32, tag="ps")
                    pstiles.append(ps0)
                for c in range(NCH):
                    for o in range(NCH):
                        mm(pstiles[o // 2], c, o, xi_t)
                for pair in range(4):
                    evac_pair(0, pstiles[pair], pair)

            def matmul_tile(t, xi_t):
                # og-outer; per-pair psum tiles so each pair evacuates as soon
                # as its own accumulation stops.
                for pair in range(4):
                    ps = psum_pool.tile([128, 2 * TT], F32, tag="ps")
                    for o in (2 * pair, 2 * pair + 1):
                        for c in range(NCH):
                            mm(ps, c, o, xi_t)
                    evac_pair(t, ps, pair)

            # ---- tile 0 in small leading pieces so the PE starts ASAP ------
            xs0 = xs_pool.tile([128, NCH * TT], F16, tag="xs")
            xi0 = xi_pool.tile([128, NCH * TT], BF16, tag="xi")
            # chunk counts per piece: first pieces small for low latency
            for c0, nc_ in ((0, 1), (1, 1), (2, 2), (4, 2), (6, 2)):
                sl = slice(c0 * TT, (c0 + nc_) * TT)
                nc.sync.dma_start(out=xs0[:, sl], in_=xs_d[0, :, sl])
                x_quant(xs0, xi0, sl)

            def x_prep(t, halves):
                xs_t = xs_pool.tile([128, NCH * TT], F16, tag="xs")
                xi_t = xi_pool.tile([128, NCH * TT], BF16, tag="xi")
                if halves:
                    for i in range(2):
                        sl = slice(i * 4 * TT, (i + 1) * 4 * TT)
                        nc.sync.dma_start(out=xs_t[:, sl], in_=xs_d[t, :, sl])
                        x_quant(xs_t, xi_t, sl)
                else:
                    nc.sync.dma_start(out=xs_t[:], in_=xs_d[t])
                    x_quant(xs_t, xi_t, slice(None))
                return xi_t

            xq_next = x_prep(1, halves=True)
            matmul_tile0(xi0)
            for t in range(1, N_TTILES):
                xq_cur = xq_next
                if t + 1 < N_TTILES:
                    xq_next = x_prep(t + 1, halves=(t + 1 == 2))
                matmul_tile(t, xq_cur)

    nc.compile()
    return nc


def _quantize_w_host(w, scale):
    """Mirror of the reference f32 quantizer for w, folded with scale*SX.

    Returns (wq2 bf16 [NCH,128,D] chunk-major k x o, beta f32 [1024])."""
    w = np.asarray(w, dtype=np.float32)
    levels = np.float32(2.0**8 - 1.0)
    step = (np.float32(0.2) - np.float32(-0.2)) / levels
    q = np.clip(w, np.float32(-0.2), np.float32(0.2))
    q = np.round((q - np.float32(-0.2)) / step).astype(np.float32)
    wq = q * step + np.float32(-0.2)  # reference-exact f32 quantized w
    s = np.float32(np.float32(np.asarray(scale, dtype=np.float32).ravel()[0]) * SX)
    w2 = (s * wq).reshape(D, D)  # [o, i]
    w2T = np.ascontiguousarray(w2.T)  # [i, o]
    w2T_bf = w2T.astype(np_bf16)
    beta = 0.5 * w2T_bf.astype(np.float64).sum(axis=0)  # [o]
    wq2 = np.ascontiguousarray(w2T_bf.reshape(NCH, 128, D))
    return wq2, beta.astype(np.float32)


def _prep_inputs(x, w, scale):
    x = np.asarray(x, dtype=np.float32).reshape(N_TOK, D)
    xs = (x * INV_SX - np.float32(0.5)).astype(np.float16)
    wq2, beta = _quantize_w_host(w, scale)
    in_maps = []
    for k in range(N_CORES):
        xk = xs[k * TOK_PER_CORE : (k + 1) * TOK_PER_CORE]  # [4096, 1024]
        # [t, tt, c, p] -> [t, p, c, tt]
        xk = xk.reshape(N_TTILES, TT, NCH, 128).transpose(0, 3, 2, 1)
        in_maps.append(
            {
                "xs": np.ascontiguousarray(xk.reshape(N_TTILES, 128, NCH * TT)),
                "wq": wq2,
            }
        )
    return in_maps, beta


def _gather_output(results, beta):
    parts = []
    for k in range(N_CORES):
        o = np.asarray(results[k]["out"]).astype(np.float32)  # [t, pair, p, og2, tt]
        # o[t, pair, p, og2, tt] -> y[t*512+tt, (pair*2+og2)*128 + p]
        o = o.transpose(0, 4, 1, 3, 2).reshape(TOK_PER_CORE, D)
        parts.append(o)
    y = np.concatenate(parts, axis=0)  # [32768, 1024]
    y += beta[None, :]
    return y.reshape(16, 2048, D)


def run(x, w, scale, trace=False, **run_kwargs):
    """Build + run on the 8 NeuronCores; returns (output, BassKernelResults)."""
    in_maps, beta = _prep_inputs(x, w, scale)
    nc = build_nc()
    res = run_bass_kernel_spmd(
        nc, in_maps, core_ids=list(range(N_CORES)), trace=trace, **run_kwargs
    )
    return _gather_output(res.results, beta), res


def kernel(x, w, scale):
    out, _ = run(x, w, scale, trace=False)
    return out


# revision 12
# speedup vs baseline: 1.1439x; 1.0059x over previous
"""Trainium2 Bass kernel for AstraloraLayer: y = (quantize(x) @ quantize(W).T) * scale.

Data-parallel across 8 NeuronCores: x sharded along the flattened token axis;
quantized weights replicated; no collectives.

Math: both quantizers are mid-rise: xq = SX*(ix+0.5), wq = SW*(iw+0.5) with
ix, iw in [-128, 127] (SX=6/255, SW=0.4/255; 128*SX-3 = SX/2 exactly).
  y[n,o] = scale * sum_k xq wq = sum_k W''[k,o] * ix[k,n] + beta[o]
with W'' = scale*SX*wq (bf16, host-precomputed mirror of the reference f32
quantizer) and beta[o] = 0.5 * sum_k bf16(W''[k,o]) (added on host).
The device moving operand is the *integer* ix in bf16 (exact); the whole
quantize+scale affine collapses into the weights.

Host ships xs = fp16(x*42.5 - 0.5); device x-quantize is 2 DVE passes in the
fp16 magic domain (M16 = 1536: ulp(v+1536) = 1 for |v| <= 511, so the fp16
output cast rounds to integer, RNE like jnp.round):
  t  = max(xs + 1536, 1408)          -> fp16 (cast rounds; 1408 = -128 clamp)
  ix = min(t, 1663) - 1536           -> bf16 exact integers
All-16-bit passes keep the DVE in its 2x mode.

Per-core device program (4096 tokens, 8 token tiles of 512):
  xs  : [8][128, 4096] fp16  tile-contiguous (8 KB/partition-line DMAs)
  wq  : [8][128, 1024] bf16  k-chunk-contiguous W''
  out : [8][4][128, 2, 512] bf16 (tile, og-pair, part, og, token) - every
        out-DMA writes one fully contiguous 256 KB block.
Per tile: 64 matmuls (og-outer, 8 k-chunks each) into 2 psum groups of 4
banks; per-og evacuation alternates ACT/DVE so the tail pair drains in
parallel; out-DMA per og-pair on the Scalar HWDGE queue.

Startup choreography: 8 warmup matmuls on a zero tile ramp the PE HAM clock
while the first x quarter lands; tile 0 is DMAd/quantized in quarters and
multiplied c-outer so the PE chases the quantize stream; tiles 1-2 are
DMAd/quantized in halves to stay ahead of the warm 216 ns/MM cadence.
x prefetch rides the Sync HWDGE queue; weights + output the Scalar queue.
"""

import numpy as np

try:
    from ml_dtypes import bfloat16 as np_bf16
except ImportError:  # pragma: no cover
    np_bf16 = None

import concourse.bass as bass
import concourse.tile as tile
from concourse import bacc, mybir
from concourse.bass_utils import run_bass_kernel_spmd

F32 = mybir.dt.float32
F16 = mybir.dt.float16
BF16 = mybir.dt.bfloat16

N_CORES = 8
D = 1024
N_TOK = 16 * 2048
TOK_PER_CORE = N_TOK // N_CORES  # 4096
TT = 512  # token tile (PSUM bank = 512 f32)
N_TTILES = TOK_PER_CORE // TT  # 8
NCH = D // 128  # 8 k-chunks / o-groups

M16 = 1536.0  # fp16 magic: ulp = 1 on [1024, 2048)

SX = np.float32(np.float32(6.0) / np.float32(255.0))
INV_SX = np.float32(42.5)  # 255/6, exact

add = mybir.AluOpType.add
amax = mybir.AluOpType.max
amin = mybir.AluOpType.min


def build_nc():
    nc = bacc.Bacc(
        "TRN2",
        target_bir_lowering=False,
        debug=False,
        num_devices=N_CORES,
    )
    xs_d = nc.dram_tensor("xs", [N_TTILES, 128, NCH * TT], F16, kind="ExternalInput")
    wq_d = nc.dram_tensor("wq", [NCH, 128, D], BF16, kind="ExternalInput")
    out_d = nc.dram_tensor(
        "out", [N_TTILES, NCH // 2, 128, 2, TT], BF16, kind="ExternalOutput"
    )

    with tile.TileContext(nc) as tc:
        with (
            tc.tile_pool(name="wq", bufs=1) as wq_pool,
            tc.tile_pool(name="consts", bufs=1) as const_pool,
            tc.tile_pool(name="xs", bufs=3) as xs_pool,
            tc.tile_pool(name="tst", bufs=2) as tst_pool,
            tc.tile_pool(name="xi", bufs=3) as xi_pool,
            tc.tile_pool(name="outsb", bufs=6) as out_pool,
            tc.tile_pool(name="psum", bufs=4, space="PSUM") as psum_pool,
        ):
            # ---- weights on the Scalar HWDGE queue (one chunk = one DMA) ---
            wq_t = wq_pool.tile([128, NCH * D], BF16)
            for c in range(NCH):
                nc.scalar.dma_start(out=wq_t[:, bass.ts(c, D)], in_=wq_d[c])

            # ---- PE warmup: ramp the HAM clock while first x tile lands ----
            warm = const_pool.tile([128, TT], BF16)
            nc.vector.memset(warm[:], 0.0)
            ps_w = psum_pool.tile([128, 2 * TT], F32, tag="ps")
            NWARM = 9
            for i in range(NWARM):
                nc.tensor.matmul(
                    ps_w[:, 0:TT], warm[:, 0:128], warm[:],
                    start=(i == 0), stop=(i == NWARM - 1),
                )
            # preload the ACT Copy table so the first evac doesn't stall
            warm16 = const_pool.tile([128, 1], BF16)
            nc.scalar.copy(warm16[:], warm[:, 0:1])

            # ---- x quantize: 2 DVE passes, all 16-bit (fp16 magic) ---------
            def x_quant(xs_t, xi_t, sl):
                t_t = tst_pool.tile([128, NCH * TT], F16, tag="tst")
                nc.vector.tensor_scalar(
                    t_t[:, sl], xs_t[:, sl], M16, M16 - 128.0, add, amax
                )
                nc.vector.tensor_scalar(
                    xi_t[:, sl], t_t[:, sl], M16 + 127.0, -M16, amin, add
                )

            def evac_pair(t, ps, pair):
                """Evacuate psum pair tile (og = 2*pair, 2*pair+1) and DMA out."""
                osb = out_pool.tile([128, 2, TT], BF16, tag="osb")
                nc.scalar.copy(osb[:, 0, :], ps[:, 0:TT])
                nc.vector.tensor_copy(osb[:, 1, :], ps[:, TT : 2 * TT])
                nc.scalar.dma_start(out=out_d[t, pair], in_=osb[:])

            def mm(ps, c, o, xi_t):
                nc.tensor.matmul(
                    ps[:, (o % 2) * TT : (o % 2) * TT + TT],
                    wq_t[:, c * D + o * 128 : c * D + o * 128 + 128],
                    xi_t[:, bass.ts(c, TT)],
                    start=(c == 0), stop=(c == NCH - 1),
                )

            def matmul_tile0(xi_t):
                # Phase 1 chases the DMA/quantize stream c-outer over all og;
                # phase 2 finishes pair by pair so evacuations (and psum
                # buffer reuse for tile 1) spread out instead of bunching.
                pstiles = []
                for _p in range(4):
                    ps0 = psum_pool.tile([128, 2 * TT], F32, tag="ps")
                    pstiles.append(ps0)
                for c in range(NCH // 2):
                    for o in range(NCH):
                        mm(pstiles[o // 2], c, o, xi_t)
                for pair in range(4):
                    for o in (2 * pair, 2 * pair + 1):
                        for c in range(NCH // 2, NCH):
                            mm(pstiles[pair], c, o, xi_t)
                    evac_pair(0, pstiles[pair], pair)

            def matmul_tile(t, xi_t):
                # og-outer; per-pair psum tiles so each pair evacuates as soon
                # as its own accumulation stops.
                for pair in range(4):
                    ps = psum_pool.tile([128, 2 * TT], F32, tag="ps")
                    for o in (2 * pair, 2 * pair + 1):
                        for c in range(NCH):
                            mm(ps, c, o, xi_t)
                    evac_pair(t, ps, pair)

            # ---- tile 0 in small leading pieces so the PE starts ASAP ------
            xs0 = xs_pool.tile([128, NCH * TT], F16, tag="xs")
            xi0 = xi_pool.tile([128, NCH * TT], BF16, tag="xi")
            # chunk counts per piece: first pieces small for low latency
            for c0, nc_ in ((0, 1), (1, 1), (2, 2), (4, 2), (6, 2)):
                sl = slice(c0 * TT, (c0 + nc_) * TT)
                nc.sync.dma_start(out=xs0[:, sl], in_=xs_d[0, :, sl])
                x_quant(xs0, xi0, sl)

            def x_prep(t, halves):
                xs_t = xs_pool.tile([128, NCH * TT], F16, tag="xs")
                xi_t = xi_pool.tile([128, NCH * TT], BF16, tag="xi")
                if halves:
                    for i in range(2):
                        sl = slice(i * 4 * TT, (i + 1) * 4 * TT)
                        nc.sync.dma_start(out=xs_t[:, sl], in_=xs_d[t, :, sl])
                        x_quant(xs_t, xi_t, sl)
                else:
                    nc.sync.dma_start(out=xs_t[:], in_=xs_d[t])
                    x_quant(xs_t, xi_t, slice(None))
                return xi_t

            xq_next = x_prep(1, halves=True)
            matmul_tile0(xi0)
            for t in range(1, N_TTILES):
                xq_cur = xq_next
                if t + 1 < N_TTILES:
                    xq_next = x_prep(t + 1, halves=(t + 1 == 2))
                matmul_tile(t, xq_cur)

    nc.compile()
    return nc


def _quantize_w_host(w, scale):
    """Mirror of the reference f32 quantizer for w, folded with scale*SX.

    Returns (wq2 bf16 [NCH,128,D] chunk-major k x o, beta f32 [1024])."""
    w = np.asarray(w, dtype=np.float32)
    levels = np.float32(2.0**8 - 1.0)
    step = (np.float32(0.2) - np.float32(-0.2)) / levels
    q = np.clip(w, np.float32(-0.2), np.float32(0.2))
    q = np.round((q - np.float32(-0.2)) / step).astype(np.float32)
    wq = q * step + np.float32(-0.2)  # reference-exact f32 quantized w
    s = np.float32(np.float32(np.asarray(scale, dtype=np.float32).ravel()[0]) * SX)
    w2 = (s * wq).reshape(D, D)  # [o, i]
    w2T = np.ascontiguousarray(w2.T)  # [i, o]
    w2T_bf = w2T.astype(np_bf16)
    beta = 0.5 * w2T_bf.astype(np.float64).sum(axis=0)  # [o]
    wq2 = np.ascontiguousarray(w2T_bf.reshape(NCH, 128, D))
    return wq2, beta.astype(np.float32)


def _prep_inputs(x, w, scale):
    x = np.asarray(x, dtype=np.float32).reshape(N_TOK, D)
    xs = (x * INV_SX - np.float32(0.5)).astype(np.float16)
    wq2, beta = _quantize_w_host(w, scale)
    in_maps = []
    for k in range(N_CORES):
        xk = xs[k * TOK_PER_CORE : (k + 1) * TOK_PER_CORE]  # [4096, 1024]
        # [t, tt, c, p] -> [t, p, c, tt]
        xk = xk.reshape(N_TTILES, TT, NCH, 128).transpose(0, 3, 2, 1)
        in_maps.append(
            {
                "xs": np.ascontiguousarray(xk.reshape(N_TTILES, 128, NCH * TT)),
                "wq": wq2,
            }
        )
    return in_maps, beta


def _gather_output(results, beta):
    parts = []
    for k in range(N_CORES):
        o = np.asarray(results[k]["out"]).astype(np.float32)  # [t, pair, p, og2, tt]
        # o[t, pair, p, og2, tt] -> y[t*512+tt, (pair*2+og2)*128 + p]
        o = o.transpose(0, 4, 1, 3, 2).reshape(TOK_PER_CORE, D)
        parts.append(o)
    y = np.concatenate(parts, axis=0)  # [32768, 1024]
    y += beta[None, :]
    return y.reshape(16, 2048, D)


def run(x, w, scale, trace=False, **run_kwargs):
    """Build + run on the 8 NeuronCores; returns (output, BassKernelResults)."""
    in_maps, beta = _prep_inputs(x, w, scale)
    nc = build_nc()
    res = run_bass_kernel_spmd(
        nc, in_maps, core_ids=list(range(N_CORES)), trace=trace, **run_kwargs
    )
    return _gather_output(res.results, beta), res


def kernel(x, w, scale):
    out, _ = run(x, w, scale, trace=False)
    return out


# revision 15
# speedup vs baseline: 1.1495x; 1.0050x over previous
"""Trainium2 Bass kernel for AstraloraLayer: y = (quantize(x) @ quantize(W).T) * scale.

Data-parallel across 8 NeuronCores: x sharded along the flattened token axis;
quantized weights replicated; no collectives.

Math: both quantizers are mid-rise: xq = SX*(ix+0.5), wq = SW*(iw+0.5) with
ix, iw in [-128, 127] (SX=6/255, SW=0.4/255; 128*SX-3 = SX/2 exactly).
  y[n,o] = scale * sum_k xq wq = sum_k W''[k,o] * ix[k,n] + beta[o]
with W'' = scale*SX*wq (bf16, host-precomputed mirror of the reference f32
quantizer) and beta[o] = 0.5 * sum_k bf16(W''[k,o]) (added on host).
The device moving operand is the *integer* ix in bf16 (exact); the whole
quantize+scale affine collapses into the weights.

Host ships xs = fp16(x*42.5 - 0.5); device x-quantize is 2 DVE passes in the
fp16 magic domain (M16 = 1536: ulp(v+1536) = 1 for |v| <= 511, so the fp16
output cast rounds to integer, RNE like jnp.round):
  t  = max(xs + 1536, 1408)          -> fp16 (cast rounds; 1408 = -128 clamp)
  ix = min(t, 1663) - 1536           -> bf16 exact integers
All-16-bit passes keep the DVE in its 2x mode.

Per-core device program (4096 tokens, 8 token tiles of 512):
  xs  : [8][128, 4096] fp16  tile-contiguous (8 KB/partition-line DMAs)
  wq  : [8][128, 1024] bf16  k-chunk-contiguous W''
  out : [8][4][128, 2, 512] bf16 (tile, og-pair, part, og, token) - every
        out-DMA writes one fully contiguous 256 KB block.
Per tile: 64 matmuls (og-outer, 8 k-chunks each) into 2 psum groups of 4
banks; per-og evacuation alternates ACT/DVE so the tail pair drains in
parallel; out-DMA per og-pair on the Scalar HWDGE queue.

Startup choreography: 8 warmup matmuls on a zero tile ramp the PE HAM clock
while the first x quarter lands; tile 0 is DMAd/quantized in quarters and
multiplied c-outer so the PE chases the quantize stream; tiles 1-2 are
DMAd/quantized in halves to stay ahead of the warm 216 ns/MM cadence.
x prefetch rides the Sync HWDGE queue; weights + output the Scalar queue.
"""

import numpy as np

try:
    from ml_dtypes import bfloat16 as np_bf16
except ImportError:  # pragma: no cover
    np_bf16 = None

import concourse.bass as bass
import concourse.tile as tile
from concourse import bacc, mybir
from concourse.bass_utils import run_bass_kernel_spmd

F32 = mybir.dt.float32
F16 = mybir.dt.float16
BF16 = mybir.dt.bfloat16

N_CORES = 8
D = 1024
N_TOK = 16 * 2048
TOK_PER_CORE = N_TOK // N_CORES  # 4096
TT = 512  # token tile (PSUM bank = 512 f32)
N_TTILES = TOK_PER_CORE // TT  # 8
NCH = D // 128  # 8 k-chunks / o-groups

M16 = 1536.0  # fp16 magic: ulp = 1 on [1024, 2048)

SX = np.float32(np.float32(6.0) / np.float32(255.0))
INV_SX = np.float32(42.5)  # 255/6, exact

add = mybir.AluOpType.add
amax = mybir.AluOpType.max
amin = mybir.AluOpType.min


def build_nc():
    nc = bacc.Bacc(
        "TRN2",
        target_bir_lowering=False,
        debug=False,
        num_devices=N_CORES,
    )
    xs_d = nc.dram_tensor("xs", [N_TTILES, 128, NCH * TT], F16, kind="ExternalInput")
    wq_d = nc.dram_tensor("wq", [NCH, 128, D], BF16, kind="ExternalInput")
    out_d = nc.dram_tensor(
        "out", [N_TTILES, NCH // 2, 128, 2, TT], BF16, kind="ExternalOutput"
    )

    with tile.TileContext(nc) as tc:
        with (
            tc.tile_pool(name="wq", bufs=1) as wq_pool,
            tc.tile_pool(name="consts", bufs=1) as const_pool,
            tc.tile_pool(name="xs", bufs=3) as xs_pool,
            tc.tile_pool(name="tst", bufs=2) as tst_pool,
            tc.tile_pool(name="xi", bufs=3) as xi_pool,
            tc.tile_pool(name="outsb", bufs=6) as out_pool,
            tc.tile_pool(name="psum", bufs=4, space="PSUM") as psum_pool,
        ):
            # ---- weights on the Scalar HWDGE queue (one chunk = one DMA) ---
            wq_t = wq_pool.tile([128, NCH * D], BF16)
            for c in range(NCH):
                nc.scalar.dma_start(out=wq_t[:, bass.ts(c, D)], in_=wq_d[c])

            # ---- PE warmup: ramp the HAM clock while first x tile lands ----
            warm = const_pool.tile([128, TT], BF16)
            nc.vector.memset(warm[:], 0.0)
            ps_w = psum_pool.tile([128, 2 * TT], F32, tag="ps")
            NWARM = 7
            for i in range(NWARM):
                nc.tensor.matmul(
                    ps_w[:, 0:TT], warm[:, 0:128], warm[:],
                    start=(i == 0), stop=(i == NWARM - 1),
                )
            # preload the ACT Copy table so the first evac doesn't stall
            warm16 = const_pool.tile([128, 1], BF16)
            nc.scalar.copy(warm16[:], warm[:, 0:1])

            # ---- x quantize: 2 DVE passes, all 16-bit (fp16 magic) ---------
            def x_quant(xs_t, xi_t, sl):
                t_t = tst_pool.tile([128, NCH * TT], F16, tag="tst")
                nc.vector.tensor_scalar(
                    t_t[:, sl], xs_t[:, sl], M16, M16 - 128.0, add, amax
                )
                nc.vector.tensor_scalar(
                    xi_t[:, sl], t_t[:, sl], M16 + 127.0, -M16, amin, add
                )

            def evac_pair(t, ps, pair):
                """Evacuate psum pair tile (og = 2*pair, 2*pair+1) and DMA out."""
                osb = out_pool.tile([128, 2, TT], BF16, tag="osb")
                nc.scalar.copy(osb[:, 0, :], ps[:, 0:TT])
                nc.vector.tensor_copy(osb[:, 1, :], ps[:, TT : 2 * TT])
                nc.scalar.dma_start(out=out_d[t, pair], in_=osb[:])

            def mm(ps, c, o, xi_t):
                nc.tensor.matmul(
                    ps[:, (o % 2) * TT : (o % 2) * TT + TT],
                    wq_t[:, c * D + o * 128 : c * D + o * 128 + 128],
                    xi_t[:, bass.ts(c, TT)],
                    start=(c == 0), stop=(c == NCH - 1),
                )

            def matmul_tile0(xi_t):
                # Phase 1 chases the DMA/quantize stream c-outer over all og;
                # phase 2 finishes pair by pair so evacuations (and psum
                # buffer reuse for tile 1) spread out instead of bunching.
                pstiles = []
                for _p in range(4):
                    ps0 = psum_pool.tile([128, 2 * TT], F32, tag="ps")
                    pstiles.append(ps0)
                for c in range(NCH // 2):
                    for o in range(NCH):
                        mm(pstiles[o // 2], c, o, xi_t)
                for pair in range(4):
                    for o in (2 * pair, 2 * pair + 1):
                        for c in range(NCH // 2, NCH):
                            mm(pstiles[pair], c, o, xi_t)
                    evac_pair(0, pstiles[pair], pair)

            def matmul_tile(t, xi_t, prep=None):
                # og-outer; per-pair psum tiles so each pair evacuates as soon
                # as its own accumulation stops. The next x_prep is emitted
                # after pair 0's evacuation so the evac CAST precedes the next
                # quantize passes in the DVE FIFO (psum handoff to tile t+1
                # must not queue behind them).
                xq_next = None
                for pair in range(4):
                    ps = psum_pool.tile([128, 2 * TT], F32, tag="ps")
                    for o in (2 * pair, 2 * pair + 1):
                        for c in range(NCH):
                            mm(ps, c, o, xi_t)
                    evac_pair(t, ps, pair)
                    if pair == 0 and prep is not None:
                        xq_next = prep()
                return xq_next

            # ---- tile 0 in small leading pieces so the PE starts ASAP ------
            xs0 = xs_pool.tile([128, NCH * TT], F16, tag="xs")
            xi0 = xi_pool.tile([128, NCH * TT], BF16, tag="xi")
            # chunk counts per piece: first pieces small for low latency
            for c0, nc_ in ((0, 1), (1, 1), (2, 2), (4, 2), (6, 2)):
                sl = slice(c0 * TT, (c0 + nc_) * TT)
                nc.sync.dma_start(out=xs0[:, sl], in_=xs_d[0, :, sl])
                x_quant(xs0, xi0, sl)

            def x_prep(t, halves):
                xs_t = xs_pool.tile([128, NCH * TT], F16, tag="xs")
                xi_t = xi_pool.tile([128, NCH * TT], BF16, tag="xi")
                if halves:
                    for i in range(2):
                        sl = slice(i * 4 * TT, (i + 1) * 4 * TT)
                        nc.sync.dma_start(out=xs_t[:, sl], in_=xs_d[t, :, sl])
                        x_quant(xs_t, xi_t, sl)
                else:
                    nc.sync.dma_start(out=xs_t[:], in_=xs_d[t])
                    x_quant(xs_t, xi_t, slice(None))
                return xi_t

            xq_next = x_prep(1, halves=True)
            matmul_tile0(xi0)
            for t in range(1, N_TTILES):
                xq_cur = xq_next
                if t + 1 < N_TTILES:
                    tn = t + 1
                    xq_next = matmul_tile(
                        t, xq_cur, prep=lambda tn=tn: x_prep(tn, halves=(tn == 2))
                    )
                else:
                    matmul_tile(t, xq_cur)

    nc.compile()
    return nc


def _quantize_w_host(w, scale):
    """Mirror of the reference f32 quantizer for w, folded with scale*SX.

    Returns (wq2 bf16 [NCH,128,D] chunk-major k x o, beta f32 [1024])."""
    w = np.asarray(w, dtype=np.float32)
    levels = np.float32(2.0**8 - 1.0)
    step = (np.float32(0.2) - np.float32(-0.2)) / levels
    q = np.clip(w, np.float32(-0.2), np.float32(0.2))
    q = np.round((q - np.float32(-0.2)) / step).astype(np.float32)
    wq = q * step + np.float32(-0.2)  # reference-exact f32 quantized w
    s = np.float32(np.float32(np.asarray(scale, dtype=np.float32).ravel()[0]) * SX)
    w2 = (s * wq).reshape(D, D)  # [o, i]
    w2T = np.ascontiguousarray(w2.T)  # [i, o]
    w2T_bf = w2T.astype(np_bf16)
    beta = 0.5 * w2T_bf.astype(np.float64).sum(axis=0)  # [o]
    wq2 = np.ascontiguousarray(w2T_bf.reshape(NCH, 128, D))
    return wq2, beta.astype(np.float32)


def _prep_inputs(x, w, scale):
    x = np.asarray(x, dtype=np.float32).reshape(N_TOK, D)
    xs = (x * INV_SX - np.float32(0.5)).astype(np.float16)
    wq2, beta = _quantize_w_host(w, scale)
    in_maps = []
    for k in range(N_CORES):
        xk = xs[k * TOK_PER_CORE : (k + 1) * TOK_PER_CORE]  # [4096, 1024]
        # [t, tt, c, p] -> [t, p, c, tt]
        xk = xk.reshape(N_TTILES, TT, NCH, 128).transpose(0, 3, 2, 1)
        in_maps.append(
            {
                "xs": np.ascontiguousarray(xk.reshape(N_TTILES, 128, NCH * TT)),
                "wq": wq2,
            }
        )
    return in_maps, beta


def _gather_output(results, beta):
    parts = []
    for k in range(N_CORES):
        o = np.asarray(results[k]["out"]).astype(np.float32)  # [t, pair, p, og2, tt]
        # o[t, pair, p, og2, tt] -> y[t*512+tt, (pair*2+og2)*128 + p]
        o = o.transpose(0, 4, 1, 3, 2).reshape(TOK_PER_CORE, D)
        parts.append(o)
    y = np.concatenate(parts, axis=0)  # [32768, 1024]
    y += beta[None, :]
    return y.reshape(16, 2048, D)


def run(x, w, scale, trace=False, **run_kwargs):
    """Build + run on the 8 NeuronCores; returns (output, BassKernelResults)."""
    in_maps, beta = _prep_inputs(x, w, scale)
    nc = build_nc()
    res = run_bass_kernel_spmd(
        nc, in_maps, core_ids=list(range(N_CORES)), trace=trace, **run_kwargs
    )
    return _gather_output(res.results, beta), res


def kernel(x, w, scale):
    out, _ = run(x, w, scale, trace=False)
    return out
